# revision 27
# baseline (speedup 1.0000x reference)
"""Trainium2 Bass kernel for nn_MistralAttention_KVmix.

Decode-step (Q=1) Mistral GQA attention with a mixed-precision KV cache:
the oldest 7168 positions of K are fake-quantized (2-bit, groups of 32
along seq per d-row) and of V (2-bit, groups of 32 along head-dim per
position); the last 1025 positions stay fp32.  RoPE on the new token,
softmax over 8193 positions, output projection.

Sharding: tensor-parallel over the 8 KV heads (1 per NeuronCore); the 4
matching query heads ride along.  hidden_states replicated; o_proj
partial sums are summed across cores on the host after the kernel.

Per-core design (v2 — large-descriptor DMA + transposed-scores):
  - K/V cache loaded "p-outer": s = 64*q + j (q = partition).  Each
    partition reads one contiguous 28KB chunk -> 32KB-class DMA
    descriptors instead of the 512B strided ones (which collapse under
    8-core descriptor contention).  The quantized prefix s < 7168 is
    exactly partitions q < 112; the fp32 tail (s >= 7168) is DMA-cast
    to bf16 straight into partitions 112..127.
  - Quantization runs in the natural layout on fp32 (exact group
    min/max + round((x-mn)*inv) via the 2^23 magic trick; the +MAGIC
    round pass runs on the Scalar engine to offload DVE).  The
    dequantized values are written as bf16.
  - K^T is built by PE-transposing the bf16 dequantized tiles (1
    cyc/col).  Scores are computed TRANSPOSED: psc[q, h] per j-tile via
    lhsT = K^T tile, rhs = q-vec.  The per-group K mn term is hoisted
    out of the j-loop (it only depends on (group, head)) and applied as
    exp(mn-part) * exp(z-part), so the softmax'd p comes out of the
    Activation engine already transposed for PV -- no p transposes.
  - V mn term is applied as 64 broadcast matmuls into the same PSUM
    accumulator as the PV value matmuls.
  - round(x) = (x + 2^23) - 2^23 (fp32 RNE == jnp.round half-to-even).
"""

import os
import sys

import numpy as np

for _p in ("/opt/trn_rl_repo",):
    if os.path.isdir(_p) and _p not in sys.path:
        sys.path.insert(0, _p)

import concourse.bass as bass
import concourse.mybir as mybir
import concourse.tile as tile
from concourse.bass_utils import run_bass_kernel_spmd

F32 = mybir.dt.float32
FP16 = mybir.dt.float16
BF16 = mybir.dt.bfloat16
AX = mybir.AxisListType
OP = mybir.AluOpType
ACTF = mybir.ActivationFunctionType

B = 4
NH = 4          # query heads per core
D = 128
S = 8192
NQ = 7168       # quantized prefix length (both K and V)
QP = 112        # NQ / 64 partitions holding the quantized prefix
JT = 64         # rows per partition (s = 64*q + j)
MAGIC = 8388608.0        # 2^23: (t + MAGIC) - MAGIC == RNE round, t in [0,4)
MAGIC16 = 1536.0         # 2^10*1.5: fp16 RNE round for t in [0,4)
INV_SQRT_D = float(1.0 / np.sqrt(np.float32(D)))
C1 = 6.28125             # Cody-Waite 2*pi split, exact in fp32
C2 = float(np.float32(2.0 * np.pi - 6.28125))
INV_2PI = float(np.float32(1.0 / (2.0 * np.pi)))


def _bc(ap, axis, n):
    """Insert a stride-0 dim of size n at position `axis`."""
    shape = list(ap.shape)
    shape.insert(axis, n)
    return ap.unsqueeze(axis).to_broadcast(tuple(shape))


def _split_multi_waits(nc):
    """The walrus build in this container encodes at most ONE semaphore wait
    per TPB instruction ("Too many sync wait commands").  Tile's sem pass
    emits several.  Split: for each instruction with N>1 waits, insert N-1
    same-engine ENGINE_NOPs before it, each carrying one wait."""
    for f in nc.m.functions:
        blocks = list(f.blocks)
        for blk in blocks:
            live = blk.instructions
            orig = list(live)
            new = []
            changed = False
            for inst in orig:
                si = inst.sync_info
                waits = list(si.on_wait) if (si and si.on_wait) else []
                if len(waits) > 1 and inst.engine != mybir.EngineType.Unassigned:
                    eng = nc.engines[inst.engine]
                    for w in waits[:-1]:
                        nop = eng.drain().ins
                        # eng.isa appended the nop to nc.cur_bb; reclaim it.
                        for b2 in f.blocks:
                            l2 = b2.instructions
                            if l2 and l2[-1] is nop:
                                l2.pop()
                                break
                        nop.sync_info = mybir.SyncInfo(on_wait=[w],
                                                       on_update=[])
                        new.append(nop)
                    inst.sync_info = mybir.SyncInfo(
                        on_wait=[waits[-1]],
                        on_update=list(si.on_update or []))
                    changed = True
                new.append(inst)
            if changed:
                live[:] = new


def build_nc():
    nc = bass.Bass()

    hidden = nc.declare_dram_parameter("hidden", [B, 4096], F32, isOutput=False)
    kp = nc.declare_dram_parameter("kp", [B, S, D], F32, isOutput=False)
    vp = nc.declare_dram_parameter("vp", [B, S, D], F32, isOutput=False)
    # weights pre-arranged on the host so each partition's DMA read is
    # one contiguous 8-16KB chunk (large descriptors)
    wq = nc.declare_dram_parameter("wq", [128, 32, NH * D], F32,
                                   isOutput=False)
    wk = nc.declare_dram_parameter("wk", [128, 32, D], F32, isOutput=False)
    wv = nc.declare_dram_parameter("wv", [128, 32, D], F32, isOutput=False)
    wo = nc.declare_dram_parameter("wo", [128, 8, NH, 512], F32,
                                   isOutput=False)
    cosp = nc.declare_dram_parameter("cosp", [128, B], F32, isOutput=False)
    sinp = nc.declare_dram_parameter("sinp", [128, B], F32, isOutput=False)
    ident = nc.declare_dram_parameter("ident", [128, 128], F32, isOutput=False)
    identb = nc.declare_dram_parameter("identb", [128, 128], BF16,
                                       isOutput=False)
    out_d = nc.declare_dram_parameter("out", [B, 4096], F32, isOutput=True)

    with tile.TileContext(nc) as tc:
        _emit(nc, tc, hidden, kp, vp, wq, wk, wv, wo, cosp, sinp, ident,
              identb, out_d)
    _split_multi_waits(nc)
    return nc


def _emit(nc, tc, hidden, kp, vp, wq, wk, wv, wo, cosp, sinp, ident,
          identb, out_d):
    from contextlib import ExitStack

    with ExitStack() as ctx:
        ec = ctx.enter_context
        singles = ec(tc.tile_pool(name="singles", bufs=1))
        raw = ec(tc.tile_pool(name="raw", bufs=2))
        kzp = ec(tc.tile_pool(name="kzp", bufs=1))
        vzp = ec(tc.tile_pool(name="vzp", bufs=2))
        ktp = ec(tc.tile_pool(name="ktp", bufs=1))
        wpool = ec(tc.tile_pool(name="wpool", bufs=2))
        stats = ec(tc.tile_pool(name="stats", bufs=2))
        misc = ec(tc.tile_pool(name="misc", bufs=2))
        ps_tr = ec(tc.tile_pool(name="ps_tr", bufs=2, space="PSUM"))
        ps_sc = ec(tc.tile_pool(name="ps_sc", bufs=2, space="PSUM"))
        ps_sm = ec(tc.tile_pool(name="ps_sm", bufs=2, space="PSUM"))
        ps_po = ec(tc.tile_pool(name="ps_po", bufs=1, space="PSUM"))

        # ---- b=0 cache DMAs first (get HBM moving before anything else) --
        def load_cache(b):
            krawt = raw.tile([QP, JT, D], F32, tag="raw", name=f"kraw{b}")
            kview = kp[b, 0:NQ, :].rearrange("(q j) d -> q j d", q=QP)
            # b=0 in quarters: the jh=0 reduces (first DVE work of the
            # kernel) start as soon as rows 0:32 land
            step = 16 if b == 0 else 32
            for j0 in range(0, JT, step):
                nc.sync.dma_start(
                    out=krawt[:, j0:j0 + step, :],
                    in_=kview[:, j0:j0 + step, :])
            kzt = kzp.tile([128, JT, D], BF16, tag="kz", name=f"kz{b}")
            # fp32 tail s in [7168, 8192) -> bf16 straight into q = 112..127
            nc.gpsimd.dma_start(
                out=kzt[QP:128],
                in_=kp[b, NQ:S, :].rearrange("(q j) d -> q (j d)", q=16))
            vrawt = raw.tile([QP, JT, D], F32, tag="raw", name=f"vraw{b}")
            vview = vp[b, 0:NQ, :].rearrange("(q j) d -> q j d", q=QP)
            for j0 in range(0, JT, 32):
                nc.sync.dma_start(out=vrawt[:, j0:j0 + 32, :],
                                  in_=vview[:, j0:j0 + 32, :])
            vzt = vzp.tile([128, JT, D], BF16, tag="vz", name=f"vz{b}")
            nc.gpsimd.dma_start(
                out=vzt[QP:128],
                in_=vp[b, NQ:S, :].rearrange("(q j) d -> q (j d)", q=16))
            return {"kraw": krawt, "kz": kzt, "vraw": vrawt, "vz": vzt}

        pre = load_cache(0)

        # ---- constants -------------------------------------------------
        ident_sb = singles.tile([128, 128], F32)
        nc.sync.dma_start(out=ident_sb[:], in_=ident[:])
        identb_sb = singles.tile([128, 128], BF16)
        nc.sync.dma_start(out=identb_sb[:], in_=identb[:])
        cosT = singles.tile([128, B], F32)
        nc.sync.dma_start(out=cosT[:], in_=cosp[:])
        sinT = singles.tile([128, B], F32)
        nc.sync.dma_start(out=sinT[:], in_=sinp[:])
        zerob = singles.tile([128, 1], F32)
        nc.vector.memset(zerob[:], 0.0)
        ones_bf = singles.tile([128, 1], BF16)
        nc.vector.memset(ones_bf[:], 1.0)

        def quant_block(b, st):
            # ======== K quantization (natural layout, fp32 exact) =======
            # group g = 2q + jh over (q<112, jh, d); 32 elems j2.
            # Engine split: Pool (gpsimd) runs the min/max trees and the
            # x-mn subtract; DVE runs the strided tree-finals, V reduces,
            # *inv, and the round+scale passes; ACT runs the +MAGIC round.
            # Tree scratch: the not-yet-written kz/vz bf16 quant regions,
            # bitcast to f32 ([q<112, 2, 8, 128] each).
            kraw, vraw = st['kraw'], st['vraw']
            k4 = kraw[:].rearrange("q (jh j2) d -> q jh j2 d", jh=2)
            # combined K+V stats tile: [:, 0:256] = K (jh, d), [:, 256:512]
            # = V (j, dg) -- one sub/scale/reciprocal pass for both
            mnA = stats.tile([QP, 512], F32, tag="mnA", bufs=1)
            mxA = stats.tile([QP, 512], F32, tag="mxA", bufs=1)
            mnK = mnA[:, 0:256].rearrange("q (jh d) -> q jh d", jh=2)
            mxK = mxA[:, 0:256].rearrange("q (jh d) -> q jh d", jh=2)
            mnV = mnA[:, 256:512].rearrange("q (j g) -> q j g", g=4)
            mxV = mxA[:, 256:512].rearrange("q (j g) -> q j g", g=4)
            for jh in range(2):
                kRv = kraw[:, 32 * jh:32 * jh + 32, :].transpose([0, 2, 1])
                nc.vector.tensor_reduce(mnK[:, jh, :], kRv, axis=AX.X,
                                        op=OP.min)
                nc.vector.tensor_reduce(mxK[:, jh, :], kRv, axis=AX.X,
                                        op=OP.max)

            # V reduces on DVE (contiguous, full speed)
            v4 = vraw[:].rearrange("q j (dg e) -> q j dg e", e=32)
            for vh in range(2):
                vsl = slice(32 * vh, 32 * vh + 32)
                nc.vector.tensor_reduce(mnV[:, vsl, :], v4[:, vsl], axis=AX.X,
                                        op=OP.min)
                nc.vector.tensor_reduce(mxV[:, vsl, :], v4[:, vsl], axis=AX.X,
                                        op=OP.max)
            scA = stats.tile([QP, 512], F32, tag="scA", bufs=1)
            nc.vector.tensor_sub(scA[:], mxA[:], mnA[:])
            nc.vector.tensor_scalar(scA[:], scA[:], 1.0 / 3.0, None, OP.mult)
            invA = stats.tile([QP, 512], F32, tag="invA", bufs=1)
            nc.vector.reciprocal(invA[:], scA[:])
            scK = scA[:, 0:256].rearrange("q (jh d) -> q jh d", jh=2)
            invK = invA[:, 0:256].rearrange("q (jh d) -> q jh d", jh=2)
            scV = scA[:, 256:512].rearrange("q (j g) -> q j g", g=4)
            invV = invA[:, 256:512].rearrange("q (j g) -> q j g", g=4)
            mnVb = stats.tile([QP, JT, 4], BF16, tag="mnVb",
                              name=f"mnVb{b}")
            st['mnVb'] = mnVb
            nc.scalar.copy(mnVb[:], mnV[:])

            # mn^T for the hoisted score term: [d, jh, q], zero for q>=112
            ps_mk = ps_tr.tile([128, 2, QP], F32, tag="tr")
            for jh in range(2):
                nc.tensor.transpose(ps_mk[:, jh, :], mnK[:, jh, :],
                                    ident_sb[0:QP, 0:QP])
            mnKTb = stats.tile([128, 2, 128], BF16, tag="mnKTb",
                               name=f"mnKTb{b}")
            st['mnKTb'] = mnKTb
            nc.gpsimd.memset(mnKTb[:], 0.0)
            nc.scalar.copy(mnKTb[:, :, 0:QP], ps_mk[:])

            # P1 on Pool (frees DVE), P2 on DVE (per jh: 3D APs are full
            # speed, merged 4D pays a ~1.6x walker penalty), P3 on ACT,
            # P4 on DVE
            for jh in range(2):
                nc.gpsimd.tensor_sub(k4[:, jh], k4[:, jh],
                                     _bc(mnK[:, jh, :], 1, 32))
            for vh in range(2):
                vsl = slice(32 * vh, 32 * vh + 32)
                nc.gpsimd.tensor_sub(v4[:, vsl], v4[:, vsl],
                                     _bc(mnV[:, vsl, :], 3, 32))
            for jh in range(2):
                khalf = kraw[:, 32 * jh:32 * jh + 32, :]
                nc.vector.tensor_mul(khalf, khalf,
                                     _bc(invK[:, jh, :], 1, 32))
            for jh in range(2):
                khj = kraw[:, 32 * jh:32 * jh + 32, :]
                nc.scalar.activation(khj, khj, ACTF.Copy, bias=MAGIC)
            for jh in range(2):
                khalf = kraw[:, 32 * jh:32 * jh + 32, :]
                nc.vector.scalar_tensor_tensor(
                    st['kz'][0:QP, 32 * jh:32 * jh + 32, :], khalf, MAGIC,
                    _bc(scK[:, jh, :], 1, 32), OP.subtract, OP.mult)



            # ======== K^T via PE transposes (bf16) ======================
            ktt = ktp.tile([128, JT, 128], BF16, tag="kt",
                           name=f"ktt{b}")
            st['ktt'] = ktt
            for g4 in range(16):
                ps_t = ps_tr.tile([128, 4, 128], BF16, tag="tr")
                for j4 in range(4):
                    nc.tensor.transpose(ps_t[:, j4, :],
                                        st['kz'][:, 4 * g4 + j4, :],
                                        identb_sb[:])
                nc.scalar.copy(ktt[:, 4 * g4:4 * g4 + 4, :].rearrange(
                    "p j d -> p (j d)"), ps_t[:].rearrange("p j d -> p (j d)"))

            # V: P2 DVE, P3 ACT, P4 DVE
            vz4 = st['vz'][0:QP].rearrange("q j (dg e) -> q j dg e", e=32)
            for vh in range(2):
                vsl = slice(32 * vh, 32 * vh + 32)
                nc.vector.tensor_mul(v4[:, vsl], v4[:, vsl],
                                     _bc(invV[:, vsl, :], 3, 32))
                vrh = vraw[:, 32 * vh:32 * vh + 32, :]
                nc.scalar.activation(vrh, vrh, ACTF.Copy, bias=MAGIC)
                nc.vector.scalar_tensor_tensor(
                    vz4[:, vsl], v4[:, vsl], MAGIC,
                    _bc(scV[:, vsl, :], 3, 32), OP.subtract, OP.mult)

        def attn_block(b, st):
            # ======== transposed scores =================================
            qb = qb_bf[:, :, b]
            ps_sT = ps_sc.tile([128, JT, NH], F32, tag="sc")
            for t in range(JT):
                nc.tensor.matmul(ps_sT[:, t, :], st['ktt'][:, t, :], qb,
                                 start=True, stop=True)
            # hoisted mn term: mnq[q, jh, h] ; emn = exp(mnq/sqrt(D))
            ps_mq = ps_sm.tile([128, 2, NH], F32, tag="sm")
            for jh in range(2):
                nc.tensor.matmul(ps_mq[:, jh, :], st['mnKTb'][:, jh, :], qb,
                                 start=True, stop=True)
            emn = misc.tile([128, 2, NH], F32, tag="emn")
            nc.scalar.activation(emn[:], ps_mq[:], ACTF.Exp,
                                 bias=zerob[:], scale=INV_SQRT_D)
            pexpf = misc.tile([128, JT, NH], F32, tag="pexpf")
            nc.scalar.activation(pexpf[:], ps_sT[:], ACTF.Exp,
                                 bias=zerob[:], scale=INV_SQRT_D)
            pT = misc.tile([128, JT + 1, NH], BF16, tag="pT")
            nc.gpsimd.memset(pT[:, JT, :], 0.0)
            nc.vector.tensor_mul(
                pT[:, 0:JT, :].rearrange("q (jh t2) h -> q jh t2 h", jh=2),
                pexpf[:].rearrange("q (jh t2) h -> q jh t2 h", jh=2),
                _bc(emn[:], 2, 32))
            # new-token column (s = 8192)
            ps_nt = ps_sm.tile([1, NH], F32, tag="nt", bufs=1)
            nc.tensor.matmul(ps_nt[:], kR_bf[:, b:b + 1], qb,
                             start=True, stop=True)
            nc.scalar.activation(pT[0:1, JT, :], ps_nt[:], ACTF.Exp,
                                 bias=zerob[0:1], scale=INV_SQRT_D)
            # denominator: ones^T @ pT summed over (q, t)
            ps_dn = ps_sm.tile([1, NH, JT + 1], F32, tag="nt", bufs=1)
            nc.tensor.matmul(ps_dn[:], ones_bf[:],
                             pT[:].transpose([0, 2, 1]),
                             start=True, stop=True)
            den_row = misc.tile([1, NH], F32, tag="den_row")
            nc.vector.tensor_reduce(den_row[:], ps_dn[:], axis=AX.X,
                                    op=OP.add)
            rsc = misc.tile([NH, 1], F32, tag="rsc")
            den_col = misc.tile([NH, 1], F32, tag="den_col")
            nc.scalar.dma_start(out=den_col[:], in_=den_row[:])
            nc.vector.reciprocal(rsc[:], den_col[:])

            # ======== PV ================================================
            po = ps_po.tile([NH, D], F32, tag="po")
            for t in range(JT):
                nc.tensor.matmul(po[:], pT[:, t, :], st['vz'][:, t, :],
                                 start=(t == 0), stop=False)
                nc.tensor.matmul(po[:], pT[0:QP, t, :],
                                 _bc(st['mnVb'][:, t, :], 2, 32),
                                 start=False, stop=False)
            nc.tensor.matmul(po[:], pT[0:1, JT, :], v_new_b[0:1, b, :],
                             start=False, stop=True)
            ob = misc.tile([NH, D], F32, tag="ob")
            nc.scalar.activation(ob[:], po[:], ACTF.Copy, scale=rsc[:])
            ps_ot = ps_sm.tile([128, NH], F32, tag="sm")
            nc.tensor.transpose(ps_ot[:], ob[:], ident_sb[0:NH, 0:NH])
            nc.scalar.copy(oTb[:, :, b], ps_ot[:])
            # per-b o_proj: out[b, :] = sum_h oTb[:, h, b]^T @ wo_h
            for half in range(2):
                for n4 in range(4):
                    nch = 4 * half + n4
                    pso = ps_sm.tile([1, 512], F32, tag="sm",
                                     name=f"pso{b}_{nch}")
                    for h in range(NH):
                        nc.tensor.matmul(pso[:], oTb[:, h, b:b + 1],
                                         wo_sb[half][:, n4, h, :],
                                         start=(h == 0), stop=(h == NH - 1))
                    outp = misc.tile([1, 512], F32, tag="outp", bufs=2,
                                     name=f"outp{b}_{nch}")
                    nc.scalar.copy(outp[:], pso[:])
                    nc.scalar.dma_start(
                        out=out_d[b:b + 1, 512 * nch:512 * (nch + 1)],
                        in_=outp[:])


        state = {0: pre}
        quant_block(0, state[0])

        # ---- hidden^T: [128 hid, 32 k, 4 b] ---------------------------
        hT = singles.tile([128, 32, B], F32)
        for kk in range(0, 32, 16):
            hst = misc.tile([B, 16 * 128], F32, tag="hst", bufs=1)
            nc.sync.dma_start(out=hst[:],
                              in_=hidden[:, 2048 * (kk // 16):
                                         2048 * (kk // 16 + 1)])
            ps_h = ps_sm.tile([128, 16 * B], F32, tag="sm")
            for j in range(16):
                nc.tensor.transpose(
                    ps_h[:, 4 * j:4 * j + 4],
                    hst[:, 128 * j:128 * (j + 1)],
                    ident_sb[0:B, 0:B],
                )
            nc.scalar.copy(hT[:, kk:kk + 16, :].rearrange("p k b -> p (k b)"),
                           ps_h[:])

        # ---- projections (wq in 4 chunks, wk/wv whole) -----------------
        q_bh = singles.tile([B, NH * D], F32)
        ps_q = ps_sm.tile([B, NH * D], F32, tag="sm")
        for c in range(4):
            wqc = wpool.tile([128, 8, NH * D], F32, tag="w", name=f"wq{c}")
            nc.sync.dma_start(out=wqc[:], in_=wq[:, 8 * c:8 * c + 8, :])
            for k8 in range(8):
                k = 8 * c + k8
                nc.tensor.matmul(ps_q[:], hT[:, k, :], wqc[:, k8, :],
                                 start=(k == 0), stop=(k == 31))
        nc.scalar.copy(q_bh[:], ps_q[:])

        k_bd = singles.tile([B, D], F32)
        v_new = singles.tile([B, D], F32)
        for w_d, dst, wtag in ((wk, k_bd, "wk"), (wv, v_new, "wv")):
            wt = wpool.tile([128, 32, D], F32, tag="w", name=wtag)
            nc.sync.dma_start(out=wt[:], in_=w_d[:])
            ps_p = ps_sm.tile([B, D], F32, tag="sm")
            for k in range(32):
                nc.tensor.matmul(ps_p[:], hT[:, k, :], wt[:, k, :],
                                 start=(k == 0), stop=(k == 31))
            nc.scalar.copy(dst[:], ps_p[:])

        # o_proj weights: two resident bf16 halves in the "w" ring
        wo_sb = []
        for half in range(2):
            wo_h = wpool.tile([128, 4, NH, 512], BF16, tag="w",
                              name=f"wo{half}")
            nc.gpsimd.dma_start(out=wo_h[:], in_=wo[:, 4 * half:4 * half + 4])
            wo_sb.append(wo_h)

        # v_new onto partition 0 as [1, B, D] then bf16
        v_new_f = singles.tile([1, B, D], F32)
        for bb in range(B):
            nc.sync.dma_start(out=v_new_f[0:1, bb, :], in_=v_new[bb:bb + 1, :])
        v_new_b = singles.tile([1, B, D], BF16)
        nc.scalar.copy(v_new_b[:], v_new_f[:])

        # transpose q -> [128 d, 4 h, 4 b], k -> [128 d, 4 b]
        ps_qT = ps_sm.tile([128, NH * B], F32, tag="sm")
        for h in range(NH):
            nc.tensor.transpose(ps_qT[:, 4 * h:4 * h + 4],
                                q_bh[:, 128 * h:128 * (h + 1)],
                                ident_sb[0:B, 0:B])
        qT = singles.tile([128, NH, B], F32)
        nc.scalar.copy(qT[:].rearrange("p h b -> p (h b)"), ps_qT[:])
        ps_kT = ps_sm.tile([128, B], F32, tag="sm")
        nc.tensor.transpose(ps_kT[:], k_bd[:], ident_sb[0:B, 0:B])
        kT = singles.tile([128, B], F32)
        nc.scalar.copy(kT[:], ps_kT[:])

        # ---- RoPE (cos/sin computed on host) --------------------------
        qsw = singles.tile([128, NH, B], F32)
        nc.sync.dma_start(out=qsw[0:64], in_=qT[64:128])
        nc.sync.dma_start(out=qsw[64:128], in_=qT[0:64])
        ksw = singles.tile([128, B], F32)
        nc.sync.dma_start(out=ksw[0:64], in_=kT[64:128])
        nc.sync.dma_start(out=ksw[64:128], in_=kT[0:64])

        qR = singles.tile([128, NH, B], F32)
        nc.vector.tensor_mul(qR[:], qT[:], _bc(cosT[:], 1, NH))
        qs2 = singles.tile([128, NH, B], F32)
        nc.vector.tensor_mul(qs2[:], qsw[:], _bc(sinT[:], 1, NH))
        nc.vector.tensor_add(qR[:], qR[:], qs2[:])
        kR = singles.tile([128, B], F32)
        nc.vector.tensor_mul(kR[:], kT[:], cosT[:])
        ks2 = singles.tile([128, B], F32)
        nc.vector.tensor_mul(ks2[:], ksw[:], sinT[:])
        nc.vector.tensor_add(kR[:], kR[:], ks2[:])

        qb_bf = singles.tile([128, NH, B], BF16)
        nc.scalar.copy(qb_bf[:], qR[:])
        kR_bf = singles.tile([128, B], BF16)
        nc.scalar.copy(kR_bf[:], kR[:])

        oTb = singles.tile([128, NH, B], BF16)

        for b in range(B):
            st = state[b]
            attn_block(b, st)
            if b + 1 < B:
                state[b + 1] = load_cache(b + 1)
                quant_block(b + 1, state[b + 1])


# ----------------------------------------------------------------------
_NC = None


def _get_nc():
    global _NC
    if _NC is None:
        _NC = build_nc()
    return _NC


def _host_consts():
    bfdt = mybir.dt.np(BF16)
    ident = np.eye(128, dtype=np.float32)
    identb = np.eye(128, dtype=np.float32).astype(bfdt)
    return ident, identb


def _host_rope(pos_f):
    """cos / sign-folded sin at the new-token positions: [128, B] f32."""
    inv_freq = (1.0 / (np.float32(10000.0) **
                       (np.arange(0, D, 2).astype(np.float32) / np.float32(D))))
    freqs = pos_f.reshape(B, 1) * inv_freq[None, :]        # [B, 64]
    emb = np.concatenate([freqs, freqs], axis=1).astype(np.float32)
    sgn = np.concatenate([-np.ones(64, np.float32), np.ones(64, np.float32)])
    cos = np.cos(emb).T                                     # [128, B]
    sin = (np.sin(emb) * sgn[None, :]).T
    return (np.ascontiguousarray(cos.astype(np.float32)),
            np.ascontiguousarray(sin.astype(np.float32)))


def _in_map(core, hid, key_past, value_past, wq, wk, wv, wo, pos_f):
    ident, identb = _host_consts()
    cosp, sinp = _host_rope(pos_f)
    # weights pre-arranged so each SBUF partition reads one contiguous
    # chunk: wq/wk/wv -> [p, k, cols] with hid = 128*k + p;
    # wo -> [p, nch, h, c] with hid = 128*h + p, col = 512*nch + c.
    wq_c = wq[:, 512 * core:512 * (core + 1)]
    wk_c = wk[:, 128 * core:128 * (core + 1)]
    wv_c = wv[:, 128 * core:128 * (core + 1)]
    wo_c = wo[512 * core:512 * (core + 1), :]
    return {
        "hidden": hid,
        "kp": np.ascontiguousarray(key_past[:, core]),
        "vp": np.ascontiguousarray(value_past[:, core]),
        "wq": np.ascontiguousarray(wq_c.reshape(32, 128, 512).swapaxes(0, 1)),
        "wk": np.ascontiguousarray(wk_c.reshape(32, 128, 128).swapaxes(0, 1)),
        "wv": np.ascontiguousarray(wv_c.reshape(32, 128, 128).swapaxes(0, 1)),
        "wo": np.ascontiguousarray(
            wo_c.reshape(4, 128, 8, 512).transpose(1, 2, 0, 3)),
        "cosp": cosp,
        "sinp": sinp,
        "ident": ident,
        "identb": identb,
    }


def kernel(hidden_states, key_past, value_past, wq, wk, wv, wo, position_ids,
           past_len):
    hidden_states = np.asarray(hidden_states, np.float32)
    key_past = np.asarray(key_past, np.float32)
    value_past = np.asarray(value_past, np.float32)
    wq = np.asarray(wq, np.float32)
    wk = np.asarray(wk, np.float32)
    wv = np.asarray(wv, np.float32)
    wo = np.asarray(wo, np.float32)
    position_ids = np.asarray(position_ids)

    nc = _get_nc()
    pos_f = position_ids.astype(np.float32).reshape(1, B)
    hid = np.ascontiguousarray(hidden_states.reshape(B, 4096))

    in_maps = [
        _in_map(c, hid, key_past, value_past, wq, wk, wv, wo, pos_f)
        for c in range(8)
    ]
    res = run_bass_kernel_spmd(nc, in_maps, list(range(8)))
    out = np.zeros((B, 4096), np.float32)
    for r in res.results:
        out = out + r["out"]
    return out.reshape(B, 1, 4096)



# revision 29
# speedup vs baseline: 1.1222x; 1.1222x over previous
"""Trainium2 Bass kernel for nn_MistralAttention_KVmix.

Decode-step (Q=1) Mistral GQA attention with a mixed-precision KV cache:
the oldest 7168 positions of K are fake-quantized (2-bit, groups of 32
along seq per d-row) and of V (2-bit, groups of 32 along head-dim per
position); the last 1025 positions stay fp32.  RoPE on the new token,
softmax over 8193 positions, output projection.

Sharding: tensor-parallel over the 8 KV heads (1 per NeuronCore); the 4
matching query heads ride along.  hidden_states replicated; o_proj
partial sums are summed across cores on the host after the kernel.

Per-core design (v2 — large-descriptor DMA + transposed-scores):
  - K/V cache loaded "p-outer": s = 64*q + j (q = partition).  Each
    partition reads one contiguous 28KB chunk -> 32KB-class DMA
    descriptors instead of the 512B strided ones (which collapse under
    8-core descriptor contention).  The quantized prefix s < 7168 is
    exactly partitions q < 112; the fp32 tail (s >= 7168) is DMA-cast
    to bf16 straight into partitions 112..127.
  - Quantization runs in the natural layout on fp32 (exact group
    min/max + round((x-mn)*inv) via the 2^23 magic trick; the +MAGIC
    round pass runs on the Scalar engine to offload DVE).  The
    dequantized values are written as bf16.
  - K^T is built by PE-transposing the bf16 dequantized tiles (1
    cyc/col).  Scores are computed TRANSPOSED: psc[q, h] per j-tile via
    lhsT = K^T tile, rhs = q-vec.  The per-group K mn term is hoisted
    out of the j-loop (it only depends on (group, head)) and applied as
    exp(mn-part) * exp(z-part), so the softmax'd p comes out of the
    Activation engine already transposed for PV -- no p transposes.
  - V mn term is applied as 64 broadcast matmuls into the same PSUM
    accumulator as the PV value matmuls.
  - round(x) = (x + 2^23) - 2^23 (fp32 RNE == jnp.round half-to-even).
"""

import os
import sys

import numpy as np

for _p in ("/opt/trn_rl_repo",):
    if os.path.isdir(_p) and _p not in sys.path:
        sys.path.insert(0, _p)

import concourse.bass as bass
import concourse.mybir as mybir
import concourse.tile as tile
from concourse.bass_utils import run_bass_kernel_spmd

F32 = mybir.dt.float32
FP16 = mybir.dt.float16
BF16 = mybir.dt.bfloat16
AX = mybir.AxisListType
OP = mybir.AluOpType
ACTF = mybir.ActivationFunctionType

B = 4
NH = 4          # query heads per core
D = 128
S = 8192
NQ = 7168       # quantized prefix length (both K and V)
QP = 112        # NQ / 64 partitions holding the quantized prefix
JT = 64         # rows per partition (s = 64*q + j)
MAGIC = 8388608.0        # 2^23: (t + MAGIC) - MAGIC == RNE round, t in [0,4)
MAGIC16 = 1536.0         # 2^10*1.5: fp16 RNE round for t in [0,4)
INV_SQRT_D = float(1.0 / np.sqrt(np.float32(D)))
C1 = 6.28125             # Cody-Waite 2*pi split, exact in fp32
C2 = float(np.float32(2.0 * np.pi - 6.28125))
INV_2PI = float(np.float32(1.0 / (2.0 * np.pi)))


def _bc(ap, axis, n):
    """Insert a stride-0 dim of size n at position `axis`."""
    shape = list(ap.shape)
    shape.insert(axis, n)
    return ap.unsqueeze(axis).to_broadcast(tuple(shape))


def _split_multi_waits(nc):
    """The walrus build in this container encodes at most ONE semaphore wait
    per TPB instruction ("Too many sync wait commands").  Tile's sem pass
    emits several.  Split: for each instruction with N>1 waits, insert N-1
    same-engine ENGINE_NOPs before it, each carrying one wait."""
    for f in nc.m.functions:
        blocks = list(f.blocks)
        for blk in blocks:
            live = blk.instructions
            orig = list(live)
            new = []
            changed = False
            for inst in orig:
                si = inst.sync_info
                waits = list(si.on_wait) if (si and si.on_wait) else []
                if len(waits) > 1 and inst.engine != mybir.EngineType.Unassigned:
                    eng = nc.engines[inst.engine]
                    for w in waits[:-1]:
                        nop = eng.drain().ins
                        # eng.isa appended the nop to nc.cur_bb; reclaim it.
                        for b2 in f.blocks:
                            l2 = b2.instructions
                            if l2 and l2[-1] is nop:
                                l2.pop()
                                break
                        nop.sync_info = mybir.SyncInfo(on_wait=[w],
                                                       on_update=[])
                        new.append(nop)
                    inst.sync_info = mybir.SyncInfo(
                        on_wait=[waits[-1]],
                        on_update=list(si.on_update or []))
                    changed = True
                new.append(inst)
            if changed:
                live[:] = new


def build_nc():
    nc = bass.Bass()

    hidden = nc.declare_dram_parameter("hidden", [B, 4096], F32, isOutput=False)
    kp = nc.declare_dram_parameter("kp", [B, S, D], F32, isOutput=False)
    vp = nc.declare_dram_parameter("vp", [B, S, D], F32, isOutput=False)
    # weights pre-arranged on the host so each partition's DMA read is
    # one contiguous 8-16KB chunk (large descriptors)
    wq = nc.declare_dram_parameter("wq", [128, 32, NH * D], F32,
                                   isOutput=False)
    wk = nc.declare_dram_parameter("wk", [128, 32, D], F32, isOutput=False)
    wv = nc.declare_dram_parameter("wv", [128, 32, D], F32, isOutput=False)
    wo = nc.declare_dram_parameter("wo", [128, 8, NH, 512], F32,
                                   isOutput=False)
    cosp = nc.declare_dram_parameter("cosp", [128, B], F32, isOutput=False)
    sinp = nc.declare_dram_parameter("sinp", [128, B], F32, isOutput=False)
    ident = nc.declare_dram_parameter("ident", [128, 128], F32, isOutput=False)
    identb = nc.declare_dram_parameter("identb", [128, 128], BF16,
                                       isOutput=False)
    out_d = nc.declare_dram_parameter("out", [B, 4096], F32, isOutput=True)

    with tile.TileContext(nc) as tc:
        _emit(nc, tc, hidden, kp, vp, wq, wk, wv, wo, cosp, sinp, ident,
              identb, out_d)
    _split_multi_waits(nc)
    return nc


def _emit(nc, tc, hidden, kp, vp, wq, wk, wv, wo, cosp, sinp, ident,
          identb, out_d):
    from contextlib import ExitStack

    with ExitStack() as ctx:
        ec = ctx.enter_context
        singles = ec(tc.tile_pool(name="singles", bufs=1))
        raw = ec(tc.tile_pool(name="raw", bufs=2))
        kzp = ec(tc.tile_pool(name="kzp", bufs=1))
        vzp = ec(tc.tile_pool(name="vzp", bufs=2))
        ktp = ec(tc.tile_pool(name="ktp", bufs=1))
        wpool = ec(tc.tile_pool(name="wpool", bufs=2))
        stats = ec(tc.tile_pool(name="stats", bufs=2))
        misc = ec(tc.tile_pool(name="misc", bufs=2))
        ps_tr = ec(tc.tile_pool(name="ps_tr", bufs=2, space="PSUM"))
        ps_sc = ec(tc.tile_pool(name="ps_sc", bufs=2, space="PSUM"))
        ps_sm = ec(tc.tile_pool(name="ps_sm", bufs=2, space="PSUM"))
        ps_po = ec(tc.tile_pool(name="ps_po", bufs=1, space="PSUM"))

        # ---- b=0 cache DMAs first (get HBM moving before anything else) --
        def load_cache(b):
            krawt = raw.tile([QP, JT, D], F32, tag="raw", name=f"kraw{b}")
            kview = kp[b, 0:NQ, :].rearrange("(q j) d -> q j d", q=QP)
            # b=0 in quarters: the jh=0 reduces (first DVE work of the
            # kernel) start as soon as rows 0:32 land
            step = 16
            for j0 in range(0, JT, step):
                nc.sync.dma_start(
                    out=krawt[:, j0:j0 + step, :],
                    in_=kview[:, j0:j0 + step, :])
            kzt = kzp.tile([128, JT, D], BF16, tag="kz", name=f"kz{b}")
            # fp32 tail s in [7168, 8192) -> bf16 straight into q = 112..127
            nc.gpsimd.dma_start(
                out=kzt[QP:128],
                in_=kp[b, NQ:S, :].rearrange("(q j) d -> q (j d)", q=16))
            vrawt = raw.tile([QP, JT, D], F32, tag="raw", name=f"vraw{b}")
            nc.sync.dma_start(
                out=vrawt[:],
                in_=vp[b, 0:NQ, :].rearrange("(q j) d -> q (j d)", q=QP))
            vzt = vzp.tile([128, JT, D], BF16, tag="vz", name=f"vz{b}")
            nc.gpsimd.dma_start(
                out=vzt[QP:128],
                in_=vp[b, NQ:S, :].rearrange("(q j) d -> q (j d)", q=16))
            return {"kraw": krawt, "kz": kzt, "vraw": vrawt, "vz": vzt}

        pre = load_cache(0)

        # ---- constants -------------------------------------------------
        ident_sb = singles.tile([128, 128], F32)
        nc.sync.dma_start(out=ident_sb[:], in_=ident[:])
        identb_sb = singles.tile([128, 128], BF16)
        nc.sync.dma_start(out=identb_sb[:], in_=identb[:])
        cosT = singles.tile([128, B], F32)
        nc.sync.dma_start(out=cosT[:], in_=cosp[:])
        sinT = singles.tile([128, B], F32)
        nc.sync.dma_start(out=sinT[:], in_=sinp[:])
        zerob = singles.tile([128, 1], F32)
        nc.vector.memset(zerob[:], 0.0)
        ones_bf = singles.tile([128, 1], BF16)
        nc.vector.memset(ones_bf[:], 1.0)

        def quant_block(b, st):
            # ======== K quantization (natural layout, fp32 exact) =======
            # group g = 2q + jh over (q<112, jh, d); 32 elems j2.
            # Engine split: Pool (gpsimd) runs the min/max trees and the
            # x-mn subtract; DVE runs the strided tree-finals, V reduces,
            # *inv, and the round+scale passes; ACT runs the +MAGIC round.
            # Tree scratch: the not-yet-written kz/vz bf16 quant regions,
            # bitcast to f32 ([q<112, 2, 8, 128] each).
            kraw, vraw = st['kraw'], st['vraw']
            k4 = kraw[:].rearrange("q (jh j2) d -> q jh j2 d", jh=2)
            # combined K+V stats tile: [:, 0:256] = K (jh, d), [:, 256:512]
            # = V (j, dg) -- one sub/scale/reciprocal pass for both
            mnA = stats.tile([QP, 512], F32, tag="mnA", bufs=1)
            mxA = stats.tile([QP, 512], F32, tag="mxA", bufs=1)
            mnK = mnA[:, 0:256].rearrange("q (jh d) -> q jh d", jh=2)
            mxK = mxA[:, 0:256].rearrange("q (jh d) -> q jh d", jh=2)
            mnV = mnA[:, 256:512].rearrange("q (j g) -> q j g", g=4)
            mxV = mxA[:, 256:512].rearrange("q (j g) -> q j g", g=4)
            for jh in range(2):
                kRv = kraw[:, 32 * jh:32 * jh + 32, :].transpose([0, 2, 1])
                nc.vector.tensor_reduce(mnK[:, jh, :], kRv, axis=AX.X,
                                        op=OP.min)
                nc.vector.tensor_reduce(mxK[:, jh, :], kRv, axis=AX.X,
                                        op=OP.max)

            # V reduces on DVE (contiguous, full speed)
            v4 = vraw[:].rearrange("q j (dg e) -> q j dg e", e=32)
            nc.vector.tensor_reduce(mnV[:], v4, axis=AX.X, op=OP.min)
            nc.vector.tensor_reduce(mxV[:], v4, axis=AX.X, op=OP.max)
            scA = stats.tile([QP, 512], F32, tag="scA", bufs=1)
            nc.vector.tensor_sub(scA[:], mxA[:], mnA[:])
            nc.vector.tensor_scalar(scA[:], scA[:], 1.0 / 3.0, None, OP.mult)
            invA = stats.tile([QP, 512], F32, tag="invA", bufs=1)
            nc.vector.reciprocal(invA[:], scA[:])
            scK = scA[:, 0:256].rearrange("q (jh d) -> q jh d", jh=2)
            invK = invA[:, 0:256].rearrange("q (jh d) -> q jh d", jh=2)
            scV = scA[:, 256:512].rearrange("q (j g) -> q j g", g=4)
            invV = invA[:, 256:512].rearrange("q (j g) -> q j g", g=4)
            mnVb = stats.tile([QP, JT, 4], BF16, tag="mnVb",
                              name=f"mnVb{b}")
            st['mnVb'] = mnVb
            nc.scalar.copy(mnVb[:], mnV[:])

            # mn^T for the hoisted score term: [d, jh, q], zero for q>=112
            ps_mk = ps_tr.tile([128, 2, QP], F32, tag="tr")
            for jh in range(2):
                nc.tensor.transpose(ps_mk[:, jh, :], mnK[:, jh, :],
                                    ident_sb[0:QP, 0:QP])
            mnKTb = stats.tile([128, 2, 128], BF16, tag="mnKTb",
                               name=f"mnKTb{b}")
            st['mnKTb'] = mnKTb
            nc.gpsimd.memset(mnKTb[:], 0.0)
            nc.scalar.copy(mnKTb[:, :, 0:QP], ps_mk[:])

            # P1 on Pool (frees DVE), P2 on DVE (per jh: 3D APs are full
            # speed, merged 4D pays a ~1.6x walker penalty), P3 on ACT,
            # P4 on DVE
            nc.gpsimd.tensor_sub(k4, k4, _bc(mnK[:], 2, 32))
            nc.gpsimd.tensor_sub(v4, v4, _bc(mnV[:], 3, 32))
            for jh in range(2):
                khalf = kraw[:, 32 * jh:32 * jh + 32, :]
                nc.vector.tensor_mul(khalf, khalf,
                                     _bc(invK[:, jh, :], 1, 32))
            nc.scalar.activation(kraw[:], kraw[:], ACTF.Copy, bias=MAGIC)
            for jh in range(2):
                khalf = kraw[:, 32 * jh:32 * jh + 32, :]
                nc.vector.scalar_tensor_tensor(
                    st['kz'][0:QP, 32 * jh:32 * jh + 32, :], khalf, MAGIC,
                    _bc(scK[:, jh, :], 1, 32), OP.subtract, OP.mult)



            # ======== K^T via PE transposes (bf16) ======================
            ktt = ktp.tile([128, JT, 128], BF16, tag="kt",
                           name=f"ktt{b}")
            st['ktt'] = ktt
            for g4 in range(16):
                ps_t = ps_tr.tile([128, 4, 128], BF16, tag="tr")
                for j4 in range(4):
                    nc.tensor.transpose(ps_t[:, j4, :],
                                        st['kz'][:, 4 * g4 + j4, :],
                                        identb_sb[:])
                nc.scalar.copy(ktt[:, 4 * g4:4 * g4 + 4, :].rearrange(
                    "p j d -> p (j d)"), ps_t[:].rearrange("p j d -> p (j d)"))

            # V: P2 DVE, P3 ACT, P4 DVE
            nc.vector.tensor_mul(v4, v4, _bc(invV[:], 3, 32))
            nc.scalar.activation(vraw[:], vraw[:], ACTF.Copy, bias=MAGIC)
            vz4 = st['vz'][0:QP].rearrange("q j (dg e) -> q j dg e", e=32)
            nc.vector.scalar_tensor_tensor(vz4, v4, MAGIC, _bc(scV[:], 3, 32),
                                           OP.subtract, OP.mult)

        def attn_block(b, st):
            # ======== transposed scores =================================
            qb = qb_bf[:, :, b]
            ps_sT = ps_sc.tile([128, JT, NH], F32, tag="sc")
            for t in range(JT):
                nc.tensor.matmul(ps_sT[:, t, :], st['ktt'][:, t, :], qb,
                                 start=True, stop=True)
            # hoisted mn term: mnq[q, jh, h] ; emn = exp(mnq/sqrt(D))
            ps_mq = ps_sm.tile([128, 2, NH], F32, tag="sm")
            for jh in range(2):
                nc.tensor.matmul(ps_mq[:, jh, :], st['mnKTb'][:, jh, :], qb,
                                 start=True, stop=True)
            emn = misc.tile([128, 2, NH], F32, tag="emn")
            nc.scalar.activation(emn[:], ps_mq[:], ACTF.Exp,
                                 bias=zerob[:], scale=INV_SQRT_D)
            pexpf = misc.tile([128, JT, NH], F32, tag="pexpf")
            nc.scalar.activation(pexpf[:], ps_sT[:], ACTF.Exp,
                                 bias=zerob[:], scale=INV_SQRT_D)
            pT = misc.tile([128, JT + 1, NH], BF16, tag="pT")
            nc.gpsimd.memset(pT[:, JT, :], 0.0)
            nc.vector.tensor_mul(
                pT[:, 0:JT, :].rearrange("q (jh t2) h -> q jh t2 h", jh=2),
                pexpf[:].rearrange("q (jh t2) h -> q jh t2 h", jh=2),
                _bc(emn[:], 2, 32))
            # new-token column (s = 8192)
            ps_nt = ps_sm.tile([1, NH], F32, tag="nt", bufs=1)
            nc.tensor.matmul(ps_nt[:], kR_bf[:, b:b + 1], qb,
                             start=True, stop=True)
            nc.scalar.activation(pT[0:1, JT, :], ps_nt[:], ACTF.Exp,
                                 bias=zerob[0:1], scale=INV_SQRT_D)
            # denominator: ones^T @ pT summed over (q, t)
            ps_dn = ps_sm.tile([1, NH, JT + 1], F32, tag="nt", bufs=1)
            nc.tensor.matmul(ps_dn[:], ones_bf[:],
                             pT[:].transpose([0, 2, 1]),
                             start=True, stop=True)
            den_row = misc.tile([1, NH], F32, tag="den_row")
            nc.vector.tensor_reduce(den_row[:], ps_dn[:], axis=AX.X,
                                    op=OP.add)
            rsc = misc.tile([NH, 1], F32, tag="rsc")
            den_col = misc.tile([NH, 1], F32, tag="den_col")
            nc.scalar.dma_start(out=den_col[:], in_=den_row[:])
            nc.vector.reciprocal(rsc[:], den_col[:])

            # ======== PV ================================================
            po = ps_po.tile([NH, D], F32, tag="po")
            for t in range(JT):
                nc.tensor.matmul(po[:], pT[:, t, :], st['vz'][:, t, :],
                                 start=(t == 0), stop=False)
                nc.tensor.matmul(po[:], pT[0:QP, t, :],
                                 _bc(st['mnVb'][:, t, :], 2, 32),
                                 start=False, stop=False)
            nc.tensor.matmul(po[:], pT[0:1, JT, :], v_new_b[0:1, b, :],
                             start=False, stop=True)
            ob = misc.tile([NH, D], F32, tag="ob")
            nc.scalar.activation(ob[:], po[:], ACTF.Copy, scale=rsc[:])
            ps_ot = ps_sm.tile([128, NH], F32, tag="sm")
            nc.tensor.transpose(ps_ot[:], ob[:], ident_sb[0:NH, 0:NH])
            nc.scalar.copy(oTb[:, :, b], ps_ot[:])
            # per-b o_proj: out[b, :] = sum_h oTb[:, h, b]^T @ wo_h
            for half in range(2):
                for n4 in range(4):
                    nch = 4 * half + n4
                    pso = ps_sm.tile([1, 512], F32, tag="sm",
                                     name=f"pso{b}_{nch}")
                    for h in range(NH):
                        nc.tensor.matmul(pso[:], oTb[:, h, b:b + 1],
                                         wo_sb[half][:, n4, h, :],
                                         start=(h == 0), stop=(h == NH - 1))
                    outp = misc.tile([1, 512], F32, tag="outp", bufs=2,
                                     name=f"outp{b}_{nch}")
                    nc.scalar.copy(outp[:], pso[:])
                    nc.scalar.dma_start(
                        out=out_d[b:b + 1, 512 * nch:512 * (nch + 1)],
                        in_=outp[:])


        state = {0: pre}
        quant_block(0, state[0])

        # ---- hidden^T: [128 hid, 32 k, 4 b] ---------------------------
        hT = singles.tile([128, 32, B], F32)
        for kk in range(0, 32, 16):
            hst = misc.tile([B, 16 * 128], F32, tag="hst", bufs=1)
            nc.sync.dma_start(out=hst[:],
                              in_=hidden[:, 2048 * (kk // 16):
                                         2048 * (kk // 16 + 1)])
            ps_h = ps_sm.tile([128, 16 * B], F32, tag="sm")
            for j in range(16):
                nc.tensor.transpose(
                    ps_h[:, 4 * j:4 * j + 4],
                    hst[:, 128 * j:128 * (j + 1)],
                    ident_sb[0:B, 0:B],
                )
            nc.scalar.copy(hT[:, kk:kk + 16, :].rearrange("p k b -> p (k b)"),
                           ps_h[:])

        # ---- projections (wq in 4 chunks, wk/wv whole) -----------------
        q_bh = singles.tile([B, NH * D], F32)
        ps_q = ps_sm.tile([B, NH * D], F32, tag="sm")
        for c in range(4):
            wqc = wpool.tile([128, 8, NH * D], F32, tag="w", name=f"wq{c}")
            nc.sync.dma_start(out=wqc[:], in_=wq[:, 8 * c:8 * c + 8, :])
            for k8 in range(8):
                k = 8 * c + k8
                nc.tensor.matmul(ps_q[:], hT[:, k, :], wqc[:, k8, :],
                                 start=(k == 0), stop=(k == 31))
        nc.scalar.copy(q_bh[:], ps_q[:])

        k_bd = singles.tile([B, D], F32)
        v_new = singles.tile([B, D], F32)
        for w_d, dst, wtag in ((wk, k_bd, "wk"), (wv, v_new, "wv")):
            wt = wpool.tile([128, 32, D], F32, tag="w", name=wtag)
            nc.sync.dma_start(out=wt[:], in_=w_d[:])
            ps_p = ps_sm.tile([B, D], F32, tag="sm")
            for k in range(32):
                nc.tensor.matmul(ps_p[:], hT[:, k, :], wt[:, k, :],
                                 start=(k == 0), stop=(k == 31))
            nc.scalar.copy(dst[:], ps_p[:])

        # o_proj weights: two resident bf16 halves in the "w" ring
        wo_sb = []
        for half in range(2):
            wo_h = wpool.tile([128, 4, NH, 512], BF16, tag="w",
                              name=f"wo{half}")
            nc.gpsimd.dma_start(out=wo_h[:], in_=wo[:, 4 * half:4 * half + 4])
            wo_sb.append(wo_h)

        # v_new onto partition 0 as [1, B, D] then bf16
        v_new_f = singles.tile([1, B, D], F32)
        for bb in range(B):
            nc.sync.dma_start(out=v_new_f[0:1, bb, :], in_=v_new[bb:bb + 1, :])
        v_new_b = singles.tile([1, B, D], BF16)
        nc.scalar.copy(v_new_b[:], v_new_f[:])

        # transpose q -> [128 d, 4 h, 4 b], k -> [128 d, 4 b]
        ps_qT = ps_sm.tile([128, NH * B], F32, tag="sm")
        for h in range(NH):
            nc.tensor.transpose(ps_qT[:, 4 * h:4 * h + 4],
                                q_bh[:, 128 * h:128 * (h + 1)],
                                ident_sb[0:B, 0:B])
        qT = singles.tile([128, NH, B], F32)
        nc.scalar.copy(qT[:].rearrange("p h b -> p (h b)"), ps_qT[:])
        ps_kT = ps_sm.tile([128, B], F32, tag="sm")
        nc.tensor.transpose(ps_kT[:], k_bd[:], ident_sb[0:B, 0:B])
        kT = singles.tile([128, B], F32)
        nc.scalar.copy(kT[:], ps_kT[:])

        # ---- RoPE (cos/sin computed on host) --------------------------
        qsw = singles.tile([128, NH, B], F32)
        nc.sync.dma_start(out=qsw[0:64], in_=qT[64:128])
        nc.sync.dma_start(out=qsw[64:128], in_=qT[0:64])
        ksw = singles.tile([128, B], F32)
        nc.sync.dma_start(out=ksw[0:64], in_=kT[64:128])
        nc.sync.dma_start(out=ksw[64:128], in_=kT[0:64])

        qR = singles.tile([128, NH, B], F32)
        nc.vector.tensor_mul(qR[:], qT[:], _bc(cosT[:], 1, NH))
        qs2 = singles.tile([128, NH, B], F32)
        nc.vector.tensor_mul(qs2[:], qsw[:], _bc(sinT[:], 1, NH))
        nc.vector.tensor_add(qR[:], qR[:], qs2[:])
        kR = singles.tile([128, B], F32)
        nc.vector.tensor_mul(kR[:], kT[:], cosT[:])
        ks2 = singles.tile([128, B], F32)
        nc.vector.tensor_mul(ks2[:], ksw[:], sinT[:])
        nc.vector.tensor_add(kR[:], kR[:], ks2[:])

        qb_bf = singles.tile([128, NH, B], BF16)
        nc.scalar.copy(qb_bf[:], qR[:])
        kR_bf = singles.tile([128, B], BF16)
        nc.scalar.copy(kR_bf[:], kR[:])

        oTb = singles.tile([128, NH, B], BF16)

        for b in range(B):
            st = state[b]
            if b + 1 < B:
                state[b + 1] = load_cache(b + 1)
            attn_block(b, st)
            if b + 1 < B:
                quant_block(b + 1, state[b + 1])


# ----------------------------------------------------------------------
_NC = None


def _get_nc():
    global _NC
    if _NC is None:
        _NC = build_nc()
    return _NC


def _host_consts():
    bfdt = mybir.dt.np(BF16)
    ident = np.eye(128, dtype=np.float32)
    identb = np.eye(128, dtype=np.float32).astype(bfdt)
    return ident, identb


def _host_rope(pos_f):
    """cos / sign-folded sin at the new-token positions: [128, B] f32."""
    inv_freq = (1.0 / (np.float32(10000.0) **
                       (np.arange(0, D, 2).astype(np.float32) / np.float32(D))))
    freqs = pos_f.reshape(B, 1) * inv_freq[None, :]        # [B, 64]
    emb = np.concatenate([freqs, freqs], axis=1).astype(np.float32)
    sgn = np.concatenate([-np.ones(64, np.float32), np.ones(64, np.float32)])
    cos = np.cos(emb).T                                     # [128, B]
    sin = (np.sin(emb) * sgn[None, :]).T
    return (np.ascontiguousarray(cos.astype(np.float32)),
            np.ascontiguousarray(sin.astype(np.float32)))


def _in_map(core, hid, key_past, value_past, wq, wk, wv, wo, pos_f):
    ident, identb = _host_consts()
    cosp, sinp = _host_rope(pos_f)
    # weights pre-arranged so each SBUF partition reads one contiguous
    # chunk: wq/wk/wv -> [p, k, cols] with hid = 128*k + p;
    # wo -> [p, nch, h, c] with hid = 128*h + p, col = 512*nch + c.
    wq_c = wq[:, 512 * core:512 * (core + 1)]
    wk_c = wk[:, 128 * core:128 * (core + 1)]
    wv_c = wv[:, 128 * core:128 * (core + 1)]
    wo_c = wo[512 * core:512 * (core + 1), :]
    return {
        "hidden": hid,
        "kp": np.ascontiguousarray(key_past[:, core]),
        "vp": np.ascontiguousarray(value_past[:, core]),
        "wq": np.ascontiguousarray(wq_c.reshape(32, 128, 512).swapaxes(0, 1)),
        "wk": np.ascontiguousarray(wk_c.reshape(32, 128, 128).swapaxes(0, 1)),
        "wv": np.ascontiguousarray(wv_c.reshape(32, 128, 128).swapaxes(0, 1)),
        "wo": np.ascontiguousarray(
            wo_c.reshape(4, 128, 8, 512).transpose(1, 2, 0, 3)),
        "cosp": cosp,
        "sinp": sinp,
        "ident": ident,
        "identb": identb,
    }


def kernel(hidden_states, key_past, value_past, wq, wk, wv, wo, position_ids,
           past_len):
    hidden_states = np.asarray(hidden_states, np.float32)
    key_past = np.asarray(key_past, np.float32)
    value_past = np.asarray(value_past, np.float32)
    wq = np.asarray(wq, np.float32)
    wk = np.asarray(wk, np.float32)
    wv = np.asarray(wv, np.float32)
    wo = np.asarray(wo, np.float32)
    position_ids = np.asarray(position_ids)

    nc = _get_nc()
    pos_f = position_ids.astype(np.float32).reshape(1, B)
    hid = np.ascontiguousarray(hidden_states.reshape(B, 4096))

    in_maps = [
        _in_map(c, hid, key_past, value_past, wq, wk, wv, wo, pos_f)
        for c in range(8)
    ]
    res = run_bass_kernel_spmd(nc, in_maps, list(range(8)))
    out = np.zeros((B, 4096), np.float32)
    for r in res.results:
        out = out + r["out"]
    return out.reshape(B, 1, 4096)



# revision 33
# speedup vs baseline: 1.1571x; 1.0311x over previous
"""Trainium2 Bass kernel for nn_MistralAttention_KVmix.

Decode-step (Q=1) Mistral GQA attention with a mixed-precision KV cache:
the oldest 7168 positions of K are fake-quantized (2-bit, groups of 32
along seq per d-row) and of V (2-bit, groups of 32 along head-dim per
position); the last 1025 positions stay fp32.  RoPE on the new token,
softmax over 8193 positions, output projection.

Sharding: tensor-parallel over the 8 KV heads (1 per NeuronCore); the 4
matching query heads ride along.  hidden_states replicated; o_proj
partial sums are summed across cores on the host after the kernel.

Per-core design (v2 — large-descriptor DMA + transposed-scores):
  - K/V cache loaded "p-outer": s = 64*q + j (q = partition).  Each
    partition reads one contiguous 28KB chunk -> 32KB-class DMA
    descriptors instead of the 512B strided ones (which collapse under
    8-core descriptor contention).  The quantized prefix s < 7168 is
    exactly partitions q < 112; the fp32 tail (s >= 7168) is DMA-cast
    to bf16 straight into partitions 112..127.
  - Quantization runs in the natural layout on fp32 (exact group
    min/max + round((x-mn)*inv) via the 2^23 magic trick; the +MAGIC
    round pass runs on the Scalar engine to offload DVE).  The
    dequantized values are written as bf16.
  - K^T is built by PE-transposing the bf16 dequantized tiles (1
    cyc/col).  Scores are computed TRANSPOSED: psc[q, h] per j-tile via
    lhsT = K^T tile, rhs = q-vec.  The per-group K mn term is hoisted
    out of the j-loop (it only depends on (group, head)) and applied as
    exp(mn-part) * exp(z-part), so the softmax'd p comes out of the
    Activation engine already transposed for PV -- no p transposes.
  - V mn term is applied as 64 broadcast matmuls into the same PSUM
    accumulator as the PV value matmuls.
  - round(x) = (x + 2^23) - 2^23 (fp32 RNE == jnp.round half-to-even).
"""

import os
import sys

import numpy as np

for _p in ("/opt/trn_rl_repo",):
    if os.path.isdir(_p) and _p not in sys.path:
        sys.path.insert(0, _p)

import concourse.bass as bass
import concourse.mybir as mybir
import concourse.tile as tile
from concourse.bass_utils import run_bass_kernel_spmd

F32 = mybir.dt.float32
FP16 = mybir.dt.float16
BF16 = mybir.dt.bfloat16
AX = mybir.AxisListType
OP = mybir.AluOpType
ACTF = mybir.ActivationFunctionType

B = 4
NH = 4          # query heads per core
D = 128
S = 8192
NQ = 7168       # quantized prefix length (both K and V)
QP = 112        # NQ / 64 partitions holding the quantized prefix
JT = 64         # rows per partition (s = 64*q + j)
MAGIC = 8388608.0        # 2^23: (t + MAGIC) - MAGIC == RNE round, t in [0,4)
MAGIC16 = 1536.0         # 2^10*1.5: fp16 RNE round for t in [0,4)
INV_SQRT_D = float(1.0 / np.sqrt(np.float32(D)))
C1 = 6.28125             # Cody-Waite 2*pi split, exact in fp32
C2 = float(np.float32(2.0 * np.pi - 6.28125))
INV_2PI = float(np.float32(1.0 / (2.0 * np.pi)))


def _bc(ap, axis, n):
    """Insert a stride-0 dim of size n at position `axis`."""
    shape = list(ap.shape)
    shape.insert(axis, n)
    return ap.unsqueeze(axis).to_broadcast(tuple(shape))


def _split_multi_waits(nc):
    """The walrus build in this container encodes at most ONE semaphore wait
    per TPB instruction ("Too many sync wait commands").  Tile's sem pass
    emits several.  Split: for each instruction with N>1 waits, insert N-1
    same-engine ENGINE_NOPs before it, each carrying one wait."""
    for f in nc.m.functions:
        blocks = list(f.blocks)
        for blk in blocks:
            live = blk.instructions
            orig = list(live)
            new = []
            changed = False
            for inst in orig:
                si = inst.sync_info
                waits = list(si.on_wait) if (si and si.on_wait) else []
                if len(waits) > 1 and inst.engine != mybir.EngineType.Unassigned:
                    eng = nc.engines[inst.engine]
                    for w in waits[:-1]:
                        nop = eng.drain().ins
                        # eng.isa appended the nop to nc.cur_bb; reclaim it.
                        for b2 in f.blocks:
                            l2 = b2.instructions
                            if l2 and l2[-1] is nop:
                                l2.pop()
                                break
                        nop.sync_info = mybir.SyncInfo(on_wait=[w],
                                                       on_update=[])
                        new.append(nop)
                    inst.sync_info = mybir.SyncInfo(
                        on_wait=[waits[-1]],
                        on_update=list(si.on_update or []))
                    changed = True
                new.append(inst)
            if changed:
                live[:] = new


def build_nc():
    nc = bass.Bass()

    hidden = nc.declare_dram_parameter("hidden", [B, 4096], F32, isOutput=False)
    kp = nc.declare_dram_parameter("kp", [B, S, D], F32, isOutput=False)
    vp = nc.declare_dram_parameter("vp", [B, S, D], F32, isOutput=False)
    # weights pre-arranged on the host so each partition's DMA read is
    # one contiguous 8-16KB chunk (large descriptors)
    wq = nc.declare_dram_parameter("wq", [128, 32, NH * D], F32,
                                   isOutput=False)
    wk = nc.declare_dram_parameter("wk", [128, 32, D], F32, isOutput=False)
    wv = nc.declare_dram_parameter("wv", [128, 32, D], F32, isOutput=False)
    wo = nc.declare_dram_parameter("wo", [128, 8, NH, 512], F32,
                                   isOutput=False)
    cosp = nc.declare_dram_parameter("cosp", [128, B], F32, isOutput=False)
    sinp = nc.declare_dram_parameter("sinp", [128, B], F32, isOutput=False)
    ident = nc.declare_dram_parameter("ident", [128, 128], F32, isOutput=False)
    identb = nc.declare_dram_parameter("identb", [128, 128], BF16,
                                       isOutput=False)
    out_d = nc.declare_dram_parameter("out", [B, 4096], F32, isOutput=True)

    with tile.TileContext(nc) as tc:
        _emit(nc, tc, hidden, kp, vp, wq, wk, wv, wo, cosp, sinp, ident,
              identb, out_d)
    _split_multi_waits(nc)
    return nc


def _emit(nc, tc, hidden, kp, vp, wq, wk, wv, wo, cosp, sinp, ident,
          identb, out_d):
    from contextlib import ExitStack

    with ExitStack() as ctx:
        ec = ctx.enter_context
        singles = ec(tc.tile_pool(name="singles", bufs=1))
        raw = ec(tc.tile_pool(name="raw", bufs=2))
        kzp = ec(tc.tile_pool(name="kzp", bufs=1))
        vzp = ec(tc.tile_pool(name="vzp", bufs=2))
        ktp = ec(tc.tile_pool(name="ktp", bufs=1))
        wpool = ec(tc.tile_pool(name="wpool", bufs=2))
        stats = ec(tc.tile_pool(name="stats", bufs=2))
        misc = ec(tc.tile_pool(name="misc", bufs=2))
        ps_tr = ec(tc.tile_pool(name="ps_tr", bufs=2, space="PSUM"))
        ps_sc = ec(tc.tile_pool(name="ps_sc", bufs=2, space="PSUM"))
        ps_sm = ec(tc.tile_pool(name="ps_sm", bufs=2, space="PSUM"))
        ps_po = ec(tc.tile_pool(name="ps_po", bufs=1, space="PSUM"))

        # ---- b=0 cache DMAs first (get HBM moving before anything else) --
        def load_cache(b):
            krawt = raw.tile([QP, JT, D], F32, tag="raw", name=f"kraw{b}")
            kview = kp[b, 0:NQ, :].rearrange("(q j) d -> q j d", q=QP)
            # b=0 in quarters: the jh=0 reduces (first DVE work of the
            # kernel) start as soon as rows 0:32 land
            step = 16 if b == 0 else 32
            for j0 in range(0, JT, step):
                nc.sync.dma_start(
                    out=krawt[:, j0:j0 + step, :],
                    in_=kview[:, j0:j0 + step, :])
            kzt = kzp.tile([128, JT, D], BF16, tag="kz", name=f"kz{b}")
            # fp32 tail s in [7168, 8192) -> bf16 straight into q = 112..127
            nc.gpsimd.dma_start(
                out=kzt[QP:128],
                in_=kp[b, NQ:S, :].rearrange("(q j) d -> q (j d)", q=16))
            vrawt = raw.tile([QP, JT, D], F32, tag="raw", name=f"vraw{b}")
            nc.sync.dma_start(
                out=vrawt[:],
                in_=vp[b, 0:NQ, :].rearrange("(q j) d -> q (j d)", q=QP))
            vzt = vzp.tile([128, JT, D], BF16, tag="vz", name=f"vz{b}")
            nc.gpsimd.dma_start(
                out=vzt[QP:128],
                in_=vp[b, NQ:S, :].rearrange("(q j) d -> q (j d)", q=16))
            return {"kraw": krawt, "kz": kzt, "vraw": vrawt, "vz": vzt}

        pre = load_cache(0)

        # ---- constants -------------------------------------------------
        ident_sb = singles.tile([128, 128], F32)
        nc.sync.dma_start(out=ident_sb[:], in_=ident[:])
        identb_sb = singles.tile([128, 128], BF16)
        nc.sync.dma_start(out=identb_sb[:], in_=identb[:])
        cosT = singles.tile([128, B], F32)
        nc.sync.dma_start(out=cosT[:], in_=cosp[:])
        sinT = singles.tile([128, B], F32)
        nc.sync.dma_start(out=sinT[:], in_=sinp[:])
        zerob = singles.tile([128, 1], F32)
        nc.vector.memset(zerob[:], 0.0)
        ones_bf = singles.tile([128, 1], BF16)
        nc.vector.memset(ones_bf[:], 1.0)

        def quant_block(b, st):
            # ======== K quantization (natural layout, fp32 exact) =======
            # group g = 2q + jh over (q<112, jh, d); 32 elems j2.
            # Engine split: Pool (gpsimd) runs the min/max trees and the
            # x-mn subtract; DVE runs the strided tree-finals, V reduces,
            # *inv, and the round+scale passes; ACT runs the +MAGIC round.
            # Tree scratch: the not-yet-written kz/vz bf16 quant regions,
            # bitcast to f32 ([q<112, 2, 8, 128] each).
            kraw, vraw = st['kraw'], st['vraw']
            k4 = kraw[:].rearrange("q (jh j2) d -> q jh j2 d", jh=2)
            # combined K+V stats tile: [:, 0:256] = K (jh, d), [:, 256:512]
            # = V (j, dg) -- one sub/scale/reciprocal pass for both
            mnA = stats.tile([QP, 512], F32, tag="mnA", bufs=1)
            mxA = stats.tile([QP, 512], F32, tag="mxA", bufs=1)
            mnK = mnA[:, 0:256].rearrange("q (jh d) -> q jh d", jh=2)
            mxK = mxA[:, 0:256].rearrange("q (jh d) -> q jh d", jh=2)
            mnV = mnA[:, 256:512].rearrange("q (j g) -> q j g", g=4)
            mxV = mxA[:, 256:512].rearrange("q (j g) -> q j g", g=4)
            for jh in range(2):
                kRv = kraw[:, 32 * jh:32 * jh + 32, :].transpose([0, 2, 1])
                nc.vector.tensor_reduce(mnK[:, jh, :], kRv, axis=AX.X,
                                        op=OP.min)
                nc.vector.tensor_reduce(mxK[:, jh, :], kRv, axis=AX.X,
                                        op=OP.max)

            # V reduces on DVE (contiguous, full speed)
            v4 = vraw[:].rearrange("q j (dg e) -> q j dg e", e=32)
            nc.vector.tensor_reduce(mnV[:], v4, axis=AX.X, op=OP.min)
            nc.vector.tensor_reduce(mxV[:], v4, axis=AX.X, op=OP.max)
            scA = stats.tile([QP, 512], F32, tag="scA", bufs=1)
            nc.vector.tensor_sub(scA[:], mxA[:], mnA[:])
            nc.vector.tensor_scalar(scA[:], scA[:], 1.0 / 3.0, None, OP.mult)
            invA = stats.tile([QP, 512], F32, tag="invA", bufs=1)
            nc.vector.reciprocal(invA[:], scA[:])
            scK = scA[:, 0:256].rearrange("q (jh d) -> q jh d", jh=2)
            invK = invA[:, 0:256].rearrange("q (jh d) -> q jh d", jh=2)
            scV = scA[:, 256:512].rearrange("q (j g) -> q j g", g=4)
            invV = invA[:, 256:512].rearrange("q (j g) -> q j g", g=4)
            mnVb = stats.tile([QP, JT, 4], BF16, tag="mnVb",
                              name=f"mnVb{b}")
            st['mnVb'] = mnVb
            nc.scalar.copy(mnVb[:], mnV[:])

            # mn^T for the hoisted score term: [d, jh, q], zero for q>=112
            ps_mk = ps_tr.tile([128, 2, QP], F32, tag="tr")
            for jh in range(2):
                nc.tensor.transpose(ps_mk[:, jh, :], mnK[:, jh, :],
                                    ident_sb[0:QP, 0:QP])
            mnKTb = stats.tile([128, 2, 128], BF16, tag="mnKTb",
                               name=f"mnKTb{b}")
            st['mnKTb'] = mnKTb
            nc.gpsimd.memset(mnKTb[:], 0.0)
            nc.scalar.copy(mnKTb[:, :, 0:QP], ps_mk[:])

            # P1 on Pool (frees DVE), P2 on DVE (per jh: 3D APs are full
            # speed, merged 4D pays a ~1.6x walker penalty), P3 on ACT,
            # P4 on DVE
            nc.gpsimd.tensor_sub(k4, k4, _bc(mnK[:], 2, 32))
            nc.gpsimd.tensor_sub(v4, v4, _bc(mnV[:], 3, 32))
            for jh in range(2):
                khalf = kraw[:, 32 * jh:32 * jh + 32, :]
                nc.vector.tensor_mul(khalf, khalf,
                                     _bc(invK[:, jh, :], 1, 32))
            nc.scalar.activation(kraw[:], kraw[:], ACTF.Copy, bias=MAGIC)
            for jh in range(2):
                khalf = kraw[:, 32 * jh:32 * jh + 32, :]
                nc.vector.scalar_tensor_tensor(
                    st['kz'][0:QP, 32 * jh:32 * jh + 32, :], khalf, MAGIC,
                    _bc(scK[:, jh, :], 1, 32), OP.subtract, OP.mult)



            # V: P2 DVE, P3 ACT, P4 DVE
            nc.vector.tensor_mul(v4, v4, _bc(invV[:], 3, 32))
            nc.scalar.activation(vraw[:], vraw[:], ACTF.Copy, bias=MAGIC)
            vz4 = st['vz'][0:QP].rearrange("q j (dg e) -> q j dg e", e=32)
            nc.vector.scalar_tensor_tensor(vz4, v4, MAGIC, _bc(scV[:], 3, 32),
                                           OP.subtract, OP.mult)

            # ======== K^T via PE transposes (bf16) ======================
            ktt = ktp.tile([128, JT, 128], BF16, tag="kt",
                           name=f"ktt{b}")
            st['ktt'] = ktt
            for g4 in range(16):
                ps_t = ps_tr.tile([128, 4, 128], BF16, tag="tr")
                for j4 in range(4):
                    nc.tensor.transpose(ps_t[:, j4, :],
                                        st['kz'][:, 4 * g4 + j4, :],
                                        identb_sb[:])
                nc.scalar.copy(ktt[:, 4 * g4:4 * g4 + 4, :].rearrange(
                    "p j d -> p (j d)"), ps_t[:].rearrange("p j d -> p (j d)"))

        def attn_block(b, st):
            # ======== transposed scores =================================
            qb = qb_bf[:, :, b]
            ps_sT = ps_sc.tile([128, JT, NH], F32, tag="sc")
            for t in range(JT):
                nc.tensor.matmul(ps_sT[:, t, :], st['ktt'][:, t, :], qb,
                                 start=True, stop=True)
            # hoisted mn term: mnq[q, jh, h] ; emn = exp(mnq/sqrt(D))
            ps_mq = ps_sm.tile([128, 2, NH], F32, tag="sm")
            for jh in range(2):
                nc.tensor.matmul(ps_mq[:, jh, :], st['mnKTb'][:, jh, :], qb,
                                 start=True, stop=True)
            emn = misc.tile([128, 2, NH], F32, tag="emn")
            nc.scalar.activation(emn[:], ps_mq[:], ACTF.Exp,
                                 bias=zerob[:], scale=INV_SQRT_D)
            pexpf = misc.tile([128, JT, NH], F32, tag="pexpf")
            nc.scalar.activation(pexpf[:], ps_sT[:], ACTF.Exp,
                                 bias=zerob[:], scale=INV_SQRT_D)
            pT = misc.tile([128, JT + 1, NH], BF16, tag="pT")
            nc.gpsimd.memset(pT[:, JT, :], 0.0)
            nc.vector.tensor_mul(
                pT[:, 0:JT, :].rearrange("q (jh t2) h -> q jh t2 h", jh=2),
                pexpf[:].rearrange("q (jh t2) h -> q jh t2 h", jh=2),
                _bc(emn[:], 2, 32))
            # new-token column (s = 8192)
            ps_nt = ps_sm.tile([1, NH], F32, tag="nt", bufs=1)
            nc.tensor.matmul(ps_nt[:], kR_bf[:, b:b + 1], qb,
                             start=True, stop=True)
            nc.scalar.activation(pT[0:1, JT, :], ps_nt[:], ACTF.Exp,
                                 bias=zerob[0:1], scale=INV_SQRT_D)
            # denominator: ones^T @ pT summed over (q, t)
            ps_dn = ps_sm.tile([1, NH, JT + 1], F32, tag="nt", bufs=1)
            nc.tensor.matmul(ps_dn[:], ones_bf[:],
                             pT[:].transpose([0, 2, 1]),
                             start=True, stop=True)
            den_row = misc.tile([1, NH], F32, tag="den_row")
            nc.vector.tensor_reduce(den_row[:], ps_dn[:], axis=AX.X,
                                    op=OP.add)
            rsc = misc.tile([NH, 1], F32, tag="rsc")
            den_col = misc.tile([NH, 1], F32, tag="den_col")
            nc.sync.dma_start(out=den_col[:], in_=den_row[:])
            nc.vector.reciprocal(rsc[:], den_col[:])

            # ======== PV ================================================
            po = ps_po.tile([NH, D], F32, tag="po")
            for t in range(JT):
                nc.tensor.matmul(po[:], pT[:, t, :], st['vz'][:, t, :],
                                 start=(t == 0), stop=False)
                nc.tensor.matmul(po[:], pT[0:QP, t, :],
                                 _bc(st['mnVb'][:, t, :], 2, 32),
                                 start=False, stop=False)
            nc.tensor.matmul(po[:], pT[0:1, JT, :], v_new_b[0:1, b, :],
                             start=False, stop=True)
            ob = misc.tile([NH, D], F32, tag="ob")
            nc.scalar.activation(ob[:], po[:], ACTF.Copy, scale=rsc[:])
            ps_ot = ps_sm.tile([128, NH], F32, tag="sm")
            nc.tensor.transpose(ps_ot[:], ob[:], ident_sb[0:NH, 0:NH])
            nc.scalar.copy(oTb[:, :, b], ps_ot[:])
            # per-b o_proj: out[b, :] = sum_h oTb[:, h, b]^T @ wo_h
            for half in range(2):
                for n4 in range(4):
                    nch = 4 * half + n4
                    pso = ps_sm.tile([1, 512], F32, tag="sm",
                                     name=f"pso{b}_{nch}")
                    for h in range(NH):
                        nc.tensor.matmul(pso[:], oTb[:, h, b:b + 1],
                                         wo_sb[half][:, n4, h, :],
                                         start=(h == 0), stop=(h == NH - 1))
                    outp = misc.tile([1, 512], F32, tag="outp", bufs=2,
                                     name=f"outp{b}_{nch}")
                    nc.scalar.copy(outp[:], pso[:])
                    nc.sync.dma_start(
                        out=out_d[b:b + 1, 512 * nch:512 * (nch + 1)],
                        in_=outp[:])


        state = {0: pre}
        quant_block(0, state[0])

        # ---- hidden^T: [128 hid, 32 k, 4 b] ---------------------------
        hT = singles.tile([128, 32, B], F32)
        for kk in range(0, 32, 16):
            hst = misc.tile([B, 16 * 128], F32, tag="hst", bufs=1)
            nc.sync.dma_start(out=hst[:],
                              in_=hidden[:, 2048 * (kk // 16):
                                         2048 * (kk // 16 + 1)])
            ps_h = ps_sm.tile([128, 16 * B], F32, tag="sm")
            for j in range(16):
                nc.tensor.transpose(
                    ps_h[:, 4 * j:4 * j + 4],
                    hst[:, 128 * j:128 * (j + 1)],
                    ident_sb[0:B, 0:B],
                )
            nc.scalar.copy(hT[:, kk:kk + 16, :].rearrange("p k b -> p (k b)"),
                           ps_h[:])

        # ---- projections (wq in 4 chunks, wk/wv whole) -----------------
        q_bh = singles.tile([B, NH * D], F32)
        ps_q = ps_sm.tile([B, NH * D], F32, tag="sm")
        for c in range(4):
            wqc = wpool.tile([128, 8, NH * D], F32, tag="w", name=f"wq{c}")
            nc.sync.dma_start(out=wqc[:], in_=wq[:, 8 * c:8 * c + 8, :])
            for k8 in range(8):
                k = 8 * c + k8
                nc.tensor.matmul(ps_q[:], hT[:, k, :], wqc[:, k8, :],
                                 start=(k == 0), stop=(k == 31))
        nc.scalar.copy(q_bh[:], ps_q[:])

        k_bd = singles.tile([B, D], F32)
        v_new = singles.tile([B, D], F32)
        for w_d, dst, wtag in ((wk, k_bd, "wk"), (wv, v_new, "wv")):
            wt = wpool.tile([128, 32, D], F32, tag="w", name=wtag)
            nc.sync.dma_start(out=wt[:], in_=w_d[:])
            ps_p = ps_sm.tile([B, D], F32, tag="sm")
            for k in range(32):
                nc.tensor.matmul(ps_p[:], hT[:, k, :], wt[:, k, :],
                                 start=(k == 0), stop=(k == 31))
            nc.scalar.copy(dst[:], ps_p[:])

        # o_proj weights: two resident bf16 halves in the "w" ring
        wo_sb = []
        for half in range(2):
            wo_h = wpool.tile([128, 4, NH, 512], BF16, tag="w",
                              name=f"wo{half}")
            nc.gpsimd.dma_start(out=wo_h[:], in_=wo[:, 4 * half:4 * half + 4])
            wo_sb.append(wo_h)

        # v_new onto partition 0 as [1, B, D] then bf16
        v_new_f = singles.tile([1, B, D], F32)
        for bb in range(B):
            nc.sync.dma_start(out=v_new_f[0:1, bb, :], in_=v_new[bb:bb + 1, :])
        v_new_b = singles.tile([1, B, D], BF16)
        nc.scalar.copy(v_new_b[:], v_new_f[:])

        # transpose q -> [128 d, 4 h, 4 b], k -> [128 d, 4 b]
        ps_qT = ps_sm.tile([128, NH * B], F32, tag="sm")
        for h in range(NH):
            nc.tensor.transpose(ps_qT[:, 4 * h:4 * h + 4],
                                q_bh[:, 128 * h:128 * (h + 1)],
                                ident_sb[0:B, 0:B])
        qT = singles.tile([128, NH, B], F32)
        nc.scalar.copy(qT[:].rearrange("p h b -> p (h b)"), ps_qT[:])
        ps_kT = ps_sm.tile([128, B], F32, tag="sm")
        nc.tensor.transpose(ps_kT[:], k_bd[:], ident_sb[0:B, 0:B])
        kT = singles.tile([128, B], F32)
        nc.scalar.copy(kT[:], ps_kT[:])

        # ---- RoPE (cos/sin computed on host) --------------------------
        qsw = singles.tile([128, NH, B], F32)
        nc.sync.dma_start(out=qsw[0:64], in_=qT[64:128])
        nc.sync.dma_start(out=qsw[64:128], in_=qT[0:64])
        ksw = singles.tile([128, B], F32)
        nc.sync.dma_start(out=ksw[0:64], in_=kT[64:128])
        nc.sync.dma_start(out=ksw[64:128], in_=kT[0:64])

        qR = singles.tile([128, NH, B], F32)
        nc.vector.tensor_mul(qR[:], qT[:], _bc(cosT[:], 1, NH))
        qs2 = singles.tile([128, NH, B], F32)
        nc.vector.tensor_mul(qs2[:], qsw[:], _bc(sinT[:], 1, NH))
        nc.vector.tensor_add(qR[:], qR[:], qs2[:])
        kR = singles.tile([128, B], F32)
        nc.vector.tensor_mul(kR[:], kT[:], cosT[:])
        ks2 = singles.tile([128, B], F32)
        nc.vector.tensor_mul(ks2[:], ksw[:], sinT[:])
        nc.vector.tensor_add(kR[:], kR[:], ks2[:])

        qb_bf = singles.tile([128, NH, B], BF16)
        nc.scalar.copy(qb_bf[:], qR[:])
        kR_bf = singles.tile([128, B], BF16)
        nc.scalar.copy(kR_bf[:], kR[:])

        oTb = singles.tile([128, NH, B], BF16)

        for b in range(B):
            st = state[b]
            attn_block(b, st)
            if b + 1 < B:
                state[b + 1] = load_cache(b + 1)
                quant_block(b + 1, state[b + 1])


# ----------------------------------------------------------------------
_NC = None


def _get_nc():
    global _NC
    if _NC is None:
        _NC = build_nc()
    return _NC


def _host_consts():
    bfdt = mybir.dt.np(BF16)
    ident = np.eye(128, dtype=np.float32)
    identb = np.eye(128, dtype=np.float32).astype(bfdt)
    return ident, identb


def _host_rope(pos_f):
    """cos / sign-folded sin at the new-token positions: [128, B] f32."""
    inv_freq = (1.0 / (np.float32(10000.0) **
                       (np.arange(0, D, 2).astype(np.float32) / np.float32(D))))
    freqs = pos_f.reshape(B, 1) * inv_freq[None, :]        # [B, 64]
    emb = np.concatenate([freqs, freqs], axis=1).astype(np.float32)
    sgn = np.concatenate([-np.ones(64, np.float32), np.ones(64, np.float32)])
    cos = np.cos(emb).T                                     # [128, B]
    sin = (np.sin(emb) * sgn[None, :]).T
    return (np.ascontiguousarray(cos.astype(np.float32)),
            np.ascontiguousarray(sin.astype(np.float32)))


def _in_map(core, hid, key_past, value_past, wq, wk, wv, wo, pos_f):
    ident, identb = _host_consts()
    cosp, sinp = _host_rope(pos_f)
    # weights pre-arranged so each SBUF partition reads one contiguous
    # chunk: wq/wk/wv -> [p, k, cols] with hid = 128*k + p;
    # wo -> [p, nch, h, c] with hid = 128*h + p, col = 512*nch + c.
    wq_c = wq[:, 512 * core:512 * (core + 1)]
    wk_c = wk[:, 128 * core:128 * (core + 1)]
    wv_c = wv[:, 128 * core:128 * (core + 1)]
    wo_c = wo[512 * core:512 * (core + 1), :]
    return {
        "hidden": hid,
        "kp": np.ascontiguousarray(key_past[:, core]),
        "vp": np.ascontiguousarray(value_past[:, core]),
        "wq": np.ascontiguousarray(wq_c.reshape(32, 128, 512).swapaxes(0, 1)),
        "wk": np.ascontiguousarray(wk_c.reshape(32, 128, 128).swapaxes(0, 1)),
        "wv": np.ascontiguousarray(wv_c.reshape(32, 128, 128).swapaxes(0, 1)),
        "wo": np.ascontiguousarray(
            wo_c.reshape(4, 128, 8, 512).transpose(1, 2, 0, 3)),
        "cosp": cosp,
        "sinp": sinp,
        "ident": ident,
        "identb": identb,
    }


def kernel(hidden_states, key_past, value_past, wq, wk, wv, wo, position_ids,
           past_len):
    hidden_states = np.asarray(hidden_states, np.float32)
    key_past = np.asarray(key_past, np.float32)
    value_past = np.asarray(value_past, np.float32)
    wq = np.asarray(wq, np.float32)
    wk = np.asarray(wk, np.float32)
    wv = np.asarray(wv, np.float32)
    wo = np.asarray(wo, np.float32)
    position_ids = np.asarray(position_ids)

    nc = _get_nc()
    pos_f = position_ids.astype(np.float32).reshape(1, B)
    hid = np.ascontiguousarray(hidden_states.reshape(B, 4096))

    in_maps = [
        _in_map(c, hid, key_past, value_past, wq, wk, wv, wo, pos_f)
        for c in range(8)
    ]
    res = run_bass_kernel_spmd(nc, in_maps, list(range(8)))
    out = np.zeros((B, 4096), np.float32)
    for r in res.results:
        out = out + r["out"]
    return out.reshape(B, 1, 4096)

